# revision 4
# baseline (speedup 1.0000x reference)
"""ESM2 contact predictor head on 8 Trainium2 NeuronCores.

Computes out[b, i, j] = sigmoid(x[b,i] @ W @ x[b,j] + bias) for
x: (8, 2050, 320) f32, W: (320, 320) f32, bias: (1,) f32.

Sharding: data-parallel over batch — core c handles batch element c.

Per-core algorithm (matmuls in float32r = full PE stream rate; PSUM
accumulates fp32; outputs written as bf16 and upcast on host):
  host:  xt = x[c].T as 3 K-slabs of 128 partitions; slab 2 holds
         d=256:320 in partitions 0:64 AND duplicated in 64:128 so the
         K=64 remainder can be row-packed (two concurrent K=64 matmuls
         in different row groups of the PE array via tile_position).
         wp likewise duplicates the slab-2 rows, and duplicates e-cols
         320:384 = 256:320 so phase 1 emits u with the same partition
         duplication for free.
  chip:  warmup matmuls release the PE clock-gate while inputs stream;
         phase 1: u = wp.T @ xt (u[e,i], e on partitions), k2 slab as
         packed K=64 pairs.
         phase 2: 16 strips of 128 i-rows x 2048 j-cols; per strip
         8 full-K matmuls + 2 packed K=64 pairs into one 4-bank PSUM
         tile -> single fused sigmoid+bias (ScalarE, N=2048) -> bf16
         SBUF -> DMA.
         tails: i-rows 2048:2050 (tile-col 0) and j-cols 2048:2050
         transposed (tile-col 32) computed concurrently via column
         tiling in one PSUM tile; 2x2 corner separately.
"""

import numpy as np

import concourse.mybir as mybir
import concourse.tile as tile
from concourse import bacc
from concourse.bass_utils import run_bass_kernel_spmd

N_CORES = 8
B, L, D = 8, 2050, 320
F32 = mybir.dt.float32
F32R = mybir.dt.float32r
BF16 = mybir.dt.bfloat16
SIG = mybir.ActivationFunctionType.Sigmoid

JM = 2048         # main strip j extent
CHUNK = 512

_cache = {}


def _build(bias_val: float):
    nc = bacc.Bacc("TRN2", target_bir_lowering=False, debug=False,
                   num_devices=N_CORES)
    xt_main_d = nc.dram_tensor("xt_main", [4, 128, 3, CHUNK], F32R,
                               kind="ExternalInput")
    xt_tail_d = nc.dram_tensor("xt_tail", [128, 3, 2], F32R,
                               kind="ExternalInput")
    w_d = nc.dram_tensor("w", [128, 3, 384], F32R, kind="ExternalInput")
    out_d = nc.dram_tensor("out", [JM, JM], BF16, kind="ExternalOutput")
    outti_d = nc.dram_tensor("out_ti", [2, L], BF16, kind="ExternalOutput")
    outtj_d = nc.dram_tensor("out_tj", [2, JM], BF16, kind="ExternalOutput")

    with tile.TileContext(nc) as tc:
        with (
            tc.tile_pool(name="persist", bufs=1) as pp,
            tc.tile_pool(name="outp", bufs=4) as outp,
            tc.tile_pool(name="psum", bufs=2, space="PSUM") as psp,
        ):
            bias_t = pp.tile([128, 1], F32)
            nc.vector.memset(bias_t[:], bias_val)

            w_sb = pp.tile([128, 3, 384], F32R)
            xt_sb = pp.tile([128, 3, L], F32R)
            u_sb = pp.tile([128, 3, L], F32R)

            nc.sync.dma_start(w_sb[:], w_d.ap())
            nc.sync.dma_start(xt_sb[:, :, 0:CHUNK], xt_main_d.ap()[0])
            nc.sync.dma_start(xt_sb[:, :, CHUNK:2 * CHUNK], xt_main_d.ap()[1])
            nc.sync.dma_start(xt_sb[:, :, 2 * CHUNK:3 * CHUNK],
                              xt_main_d.ap()[2])
            nc.sync.dma_start(xt_sb[:, :, 3 * CHUNK:4 * CHUNK],
                              xt_main_d.ap()[3])
            nc.sync.dma_start(xt_sb[:, :, JM:L], xt_tail_d.ap())

            # PE warmup: release the HAM clock-gate during the input-DMA
            # window; also preload the sigmoid activation table.
            warm_sb = pp.tile([128, 512], F32R)
            nc.vector.memset(warm_sb.bitcast(F32)[:], 1.0)
            psw = psp.tile([128, 2048], F32, tag="big", bufs=2, name="psw")
            for wi in range(11):
                c0 = (wi % 4) * 512
                nc.tensor.matmul(psw[:, c0:c0 + 512], lhsT=warm_sb[:, :128],
                                 rhs=warm_sb[:], start=True, stop=True)
            act_warm = pp.tile([128, 1], F32)
            nc.scalar.activation(act_warm[:], bias_t[:], SIG)

            # ---- phase 1: u[e, i] = sum_d wp[d, e] xt[d, i] ----
            def ph1_half(h):
                i0 = h * 1024
                for et in range(3):
                    e0 = et * 128
                    ps1 = psp.tile([128, 2048], F32, tag="big", bufs=2,
                                   name="ps1")
                    for k in range(2):
                        for c in range(2):
                            a = i0 + c * 512
                            nc.tensor.matmul(
                                ps1[:, c * 512:(c + 1) * 512],
                                lhsT=w_sb[:, k, e0:e0 + 128],
                                rhs=xt_sb[:, k, a:a + 512],
                                start=(k == 0), stop=False)
                    # K=64 remainder as one packed pair (row groups 0/1)
                    nc.tensor.matmul(ps1[:, 0:512],
                                     lhsT=w_sb[0:64, 2, e0:e0 + 128],
                                     rhs=xt_sb[0:64, 2, i0:i0 + 512],
                                     start=False, stop=True)
                    nc.tensor.matmul(ps1[:, 512:1024],
                                     lhsT=w_sb[64:128, 2, e0:e0 + 128],
                                     rhs=xt_sb[64:128, 2, i0 + 512:i0 + 1024],
                                     start=False, stop=True)
                    nc.vector.tensor_copy(u_sb[:, et, i0:i0 + 1024],
                                          ps1[:, 0:1024])

            ph1_half(0)
            ph1_half(1)

            # phase-1 i-tail (i = 2048:2050) + 2x2 corner
            psT = psp.tile([128, 2048], F32, tag="big", bufs=2, name="psT")
            for et in range(3):
                e0 = et * 128
                col = et * 512
                for k in range(2):
                    nc.tensor.matmul(psT[:, col:col + 2],
                                     lhsT=w_sb[:, k, e0:e0 + 128],
                                     rhs=xt_sb[:, k, JM:L],
                                     start=(k == 0), stop=False)
                nc.tensor.matmul(psT[:, col:col + 2],
                                 lhsT=w_sb[0:64, 2, e0:e0 + 128],
                                 rhs=xt_sb[0:64, 2, JM:L],
                                 start=False, stop=True)
                nc.vector.tensor_copy(u_sb[:, et, JM:L], psT[:, col:col + 2])

            tail_i = pp.tile([2, L], BF16)
            tail_j = pp.tile([2, JM], BF16)

            # corner out[2048:2050, 2048:2050] (needs u i-tail)
            for k in range(2):
                nc.tensor.matmul(psT[0:2, 1536:1538],
                                 lhsT=u_sb[:, k, JM:L],
                                 rhs=xt_sb[:, k, JM:L],
                                 start=(k == 0), stop=False)
            nc.tensor.matmul(psT[0:2, 1536:1538],
                             lhsT=u_sb[0:64, 2, JM:L],
                             rhs=xt_sb[0:64, 2, JM:L],
                             start=False, stop=True)
            nc.scalar.activation(tail_i[:, JM:L], psT[0:2, 1536:1538], SIG,
                                 bias=bias_t[0:2, :])

            # ---- tails: i-rows 2048:2050 over j, and j-cols 2048:2050
            # transposed over i. (Column tiling rejected by the ISA's
            # mm_valid_dst_partition check, so these run sequentially.)
            def tail_block(name, lhs_of, rhs_of, dest, act_sb):
                psX = psp.tile([128, 2048], F32, tag="big", bufs=2,
                               name=name)
                for k in range(2):
                    for c in range(4):
                        a = c * 512
                        nc.tensor.matmul(psX[0:2, a:a + 512],
                                         lhsT=lhs_of(k, slice(0, 128)),
                                         rhs=rhs_of(k, slice(0, 128), a),
                                         start=(k == 0), stop=False)
                for q in range(2):
                    aA = q * 1024
                    aB = q * 1024 + 512
                    nc.tensor.matmul(psX[0:2, aA:aA + 512],
                                     lhsT=lhs_of(2, slice(0, 64)),
                                     rhs=rhs_of(2, slice(0, 64), aA),
                                     start=False, stop=True)
                    nc.tensor.matmul(psX[0:2, aB:aB + 512],
                                     lhsT=lhs_of(2, slice(64, 128)),
                                     rhs=rhs_of(2, slice(64, 128), aB),
                                     start=False, stop=True)
                nc.scalar.activation(act_sb, psX[0:2, :], SIG,
                                     bias=bias_t[0:2, :])

            tail_block("psA",
                       lambda k, p: u_sb[p, k, JM:L],
                       lambda k, p, a: xt_sb[p, k, a:a + 512],
                       outti_d, tail_i[:, 0:JM])
            tail_block("psB",
                       lambda k, p: xt_sb[p, k, JM:L],
                       lambda k, p, a: u_sb[p, k, a:a + 512],
                       outtj_d, tail_j[:])
            nc.sync.dma_start(outti_d.ap()[:], tail_i[:])
            nc.sync.dma_start(outtj_d.ap()[:], tail_j[:])

            # ---- phase 2: 16 strips of 128 i-rows x 2048 j-cols ----
            for s in range(16):
                i0 = s * 128
                ps = psp.tile([128, 2048], F32, tag="big", bufs=2, name="ps")
                for k in range(2):
                    u_k = u_sb[:, k, i0:i0 + 128]
                    for c in range(4):
                        a = c * 512
                        nc.tensor.matmul(ps[:, a:a + 512], lhsT=u_k,
                                         rhs=xt_sb[:, k, a:a + 512],
                                         start=(k == 0), stop=False)
                uA = u_sb[0:64, 2, i0:i0 + 128]
                uB = u_sb[64:128, 2, i0:i0 + 128]
                for q in range(2):
                    aA = q * 1024
                    aB = q * 1024 + 512
                    nc.tensor.matmul(ps[:, aA:aA + 512], lhsT=uA,
                                     rhs=xt_sb[0:64, 2, aA:aA + 512],
                                     start=False, stop=True)
                    nc.tensor.matmul(ps[:, aB:aB + 512], lhsT=uB,
                                     rhs=xt_sb[64:128, 2, aB:aB + 512],
                                     start=False, stop=True)
                # ScalarE -> SBUF bf16 at 128 partitions is broken on this
                # silicon (even columns come back 4-bit quantized), so the
                # sigmoid writes f32 and the SWDGE DMA casts f32->bf16.
                ob = outp.tile([128, 2048], F32, tag="strip", bufs=3,
                               name="ob")
                nc.scalar.activation(ob[:], ps[:], SIG, bias=bias_t[:])
                nc.gpsimd.dma_start(out_d.ap()[i0:i0 + 128, :], ob[:])

    nc.compile()
    return nc


last_results = None


def _host_pack(x, W):
    xT = x.transpose(0, 2, 1)  # (B, 320, 2050)
    full = np.empty((B, 128, 3, L), np.float32)
    full[:, :, 0, :] = xT[:, 0:128]
    full[:, :, 1, :] = xT[:, 128:256]
    full[:, 0:64, 2, :] = xT[:, 256:320]
    full[:, 64:128, 2, :] = xT[:, 256:320]   # row-group duplicate
    xt_main = np.ascontiguousarray(
        full[..., :JM].reshape(B, 128, 3, 4, CHUNK)
        .transpose(0, 3, 1, 2, 4))
    xt_tail = np.ascontiguousarray(full[..., JM:L])
    Wp = np.zeros((384, 384), np.float32)
    Wp[0:320, 0:320] = W
    Wp[0:320, 320:384] = W[:, 256:320]       # e-col duplicate -> u row dup
    wp = np.empty((128, 3, 384), np.float32)
    wp[:, 0, :] = Wp[0:128]
    wp[:, 1, :] = Wp[128:256]
    wp[0:64, 2, :] = Wp[256:320]
    wp[64:128, 2, :] = Wp[256:320]           # row-group duplicate
    return xt_main, xt_tail, wp


def kernel(x, W, b, _trace=False):
    global last_results
    x = np.ascontiguousarray(np.asarray(x, dtype=np.float32))
    W = np.asarray(W, dtype=np.float32)
    b = np.asarray(b, dtype=np.float32)
    bias_val = float(b[0])

    if bias_val not in _cache:
        _cache.clear()
        _cache[bias_val] = _build(bias_val)
    nc = _cache[bias_val]

    xt_main, xt_tail, wp = _host_pack(x, W)
    in_maps = [{"xt_main": xt_main[c], "xt_tail": xt_tail[c], "w": wp}
               for c in range(N_CORES)]
    res = run_bass_kernel_spmd(nc, in_maps, core_ids=list(range(N_CORES)),
                               trace=_trace)
    last_results = res
    out = np.empty((B, L, L), dtype=np.float32)
    for c in range(N_CORES):
        r = res.results[c]
        out[c, :JM, :JM] = np.asarray(r["out"]).astype(np.float32)
        out[c, JM:, :] = np.asarray(r["out_ti"]).astype(np.float32)
        out[c, :JM, JM:] = np.asarray(r["out_tj"]).astype(np.float32).T
    return out


# revision 5
# speedup vs baseline: 1.3213x; 1.3213x over previous
"""ESM2 contact predictor head on 8 Trainium2 NeuronCores.

Computes out[b, i, j] = sigmoid(x[b,i] @ W @ x[b,j] + bias) for
x: (8, 2050, 320) f32, W: (320, 320) f32, bias: (1,) f32.

Sharding: data-parallel over batch — core c handles batch element c.

The chip computes only the 2048x2048 main block per batch element; the
2-row/2-col tails (4100 outputs, ~50 MFLOP total) are computed on the
host, which removes all M=2 matmul waste from the PE stream.

Per-core algorithm (matmuls in float32r = full PE stream rate; PSUM
accumulates fp32):
  host:  xt = x[c,:2048].T-ish as 3 K-slabs of 128 partitions; slab 2
         holds d=256:320 in partitions 0:64 AND duplicated in 64:128 so
         the K=64 remainder runs as row-packed pairs (two concurrent
         K=64 matmuls in different PE row groups via tile_position).
         wp duplicates slab-2 rows likewise, and duplicates e-cols
         320:384 = 256:320 so phase 1 emits u with the same partition
         duplication for free.
  chip:  warmup matmuls release the HAM clock-gate while inputs stream;
         phase 1: u = wp.T @ xt (u[e,i], e on partitions).
         phase 2: 16 strips of 128 i-rows x 2048 j-cols; 8 full-K
         matmuls + 2 packed K=64 pairs into a 4-bank PSUM tile ->
         sigmoid+bias on ScalarE (f32; the ScalarE bf16 SBUF write path
         is broken at 128 partitions on this silicon) -> DVE cast to
         bf16 -> HWDGE DMA out. Host upcasts to f32.
"""

import numpy as np

import concourse.mybir as mybir
import concourse.tile as tile
from concourse import bacc
from concourse.bass_utils import run_bass_kernel_spmd

N_CORES = 8
B, L, D = 8, 2050, 320
F32 = mybir.dt.float32
F32R = mybir.dt.float32r
BF16 = mybir.dt.bfloat16
SIG = mybir.ActivationFunctionType.Sigmoid

JM = 2048
CHUNK = 512

_cache = {}


def _build(bias_val: float):
    nc = bacc.Bacc("TRN2", target_bir_lowering=False, debug=False,
                   num_devices=N_CORES)
    xt_main_d = nc.dram_tensor("xt_main", [4, 128, 3, CHUNK], F32R,
                               kind="ExternalInput")
    w_d = nc.dram_tensor("w", [128, 3, 384], F32R, kind="ExternalInput")
    out_d = nc.dram_tensor("out", [JM, JM], BF16, kind="ExternalOutput")

    with tile.TileContext(nc) as tc:
        with (
            tc.tile_pool(name="persist", bufs=1) as pp,
            tc.tile_pool(name="outp", bufs=3) as outp,
            tc.tile_pool(name="psum", bufs=2, space="PSUM") as psp,
        ):
            bias_t = pp.tile([128, 1], F32)
            nc.vector.memset(bias_t[:], bias_val)

            w_sb = pp.tile([128, 3, 384], F32R)
            xt_sb = pp.tile([128, 3, JM], F32R)
            u_sb = pp.tile([128, 3, JM], F32R)

            nc.sync.dma_start(w_sb[:], w_d.ap())
            for c in range(4):
                nc.sync.dma_start(xt_sb[:, :, c * CHUNK:(c + 1) * CHUNK],
                                  xt_main_d.ap()[c])

            # PE warmup during the input-DMA window + sigmoid table preload
            warm_sb = pp.tile([128, 512], F32R)
            nc.vector.memset(warm_sb.bitcast(F32)[:], 1.0)
            psw = psp.tile([128, 2048], F32, tag="big", bufs=2, name="psw")
            for wi in range(11):
                c0 = (wi % 4) * 512
                nc.tensor.matmul(psw[:, c0:c0 + 512], lhsT=warm_sb[:, :128],
                                 rhs=warm_sb[:], start=True, stop=True)
            act_warm = pp.tile([128, 1], F32)
            nc.scalar.activation(act_warm[:], bias_t[:], SIG)

            # ---- phase 1: u[e, i] = sum_d wp[d, e] xt[d, i] ----
            for h in range(2):
                i0 = h * 1024
                for et in range(3):
                    e0 = et * 128
                    ps1 = psp.tile([128, 2048], F32, tag="big", bufs=2,
                                   name="ps1")
                    for k in range(2):
                        for c in range(2):
                            a = i0 + c * 512
                            nc.tensor.matmul(
                                ps1[:, c * 512:(c + 1) * 512],
                                lhsT=w_sb[:, k, e0:e0 + 128],
                                rhs=xt_sb[:, k, a:a + 512],
                                start=(k == 0), stop=False)
                    nc.tensor.matmul(ps1[:, 0:512],
                                     lhsT=w_sb[0:64, 2, e0:e0 + 128],
                                     rhs=xt_sb[0:64, 2, i0:i0 + 512],
                                     start=False, stop=True)
                    nc.tensor.matmul(ps1[:, 512:1024],
                                     lhsT=w_sb[64:128, 2, e0:e0 + 128],
                                     rhs=xt_sb[64:128, 2, i0 + 512:i0 + 1024],
                                     start=False, stop=True)
                    nc.vector.tensor_copy(u_sb[:, et, i0:i0 + 1024],
                                          ps1[:, 0:1024])

            # ---- phase 2: 16 strips of 128 i-rows x 2048 j-cols ----
            def strip_mms(i0):
                ps = psp.tile([128, 2048], F32, tag="big", bufs=2, name="ps")
                for k in range(2):
                    u_k = u_sb[:, k, i0:i0 + 128]
                    for c in range(4):
                        a = c * 512
                        nc.tensor.matmul(ps[:, a:a + 512], lhsT=u_k,
                                         rhs=xt_sb[:, k, a:a + 512],
                                         start=(k == 0), stop=False)
                uA = u_sb[0:64, 2, i0:i0 + 128]
                uB = u_sb[64:128, 2, i0:i0 + 128]
                for q in range(2):
                    aA = q * 1024
                    aB = q * 1024 + 512
                    nc.tensor.matmul(ps[:, aA:aA + 512], lhsT=uA,
                                     rhs=xt_sb[0:64, 2, aA:aA + 512],
                                     start=False, stop=True)
                    nc.tensor.matmul(ps[:, aB:aB + 512], lhsT=uB,
                                     rhs=xt_sb[64:128, 2, aB:aB + 512],
                                     start=False, stop=True)
                return ps

            def emit_out(ps, i0, a, n):
                stage = outp.tile([128, 2048], F32, tag="stage", bufs=3,
                                  name="stage")
                ob = outp.tile([128, 2048], BF16, tag="strip", bufs=3,
                               name="ob")
                nc.scalar.activation(stage[:, a:a + n], ps[:, a:a + n], SIG,
                                     bias=bias_t[:])
                nc.vector.tensor_copy(ob[:, a:a + n], stage[:, a:a + n])
                nc.sync.dma_start(out_d.ap()[i0:i0 + 128, a:a + n],
                                  ob[:, a:a + n])

            for s in range(15):
                ps = strip_mms(s * 128)
                emit_out(ps, s * 128, 0, 2048)
            # last strip: two output halves so the end-of-kernel drain is
            # one half-strip deep instead of a full strip
            ps = strip_mms(15 * 128)
            emit_out(ps, 15 * 128, 0, 1024)
            emit_out(ps, 15 * 128, 1024, 1024)

    nc.compile()
    return nc


last_results = None


def _host_pack(x, W):
    xT = x[:, :JM].transpose(0, 2, 1)  # (B, 320, 2048)
    full = np.empty((B, 128, 3, JM), np.float32)
    full[:, :, 0, :] = xT[:, 0:128]
    full[:, :, 1, :] = xT[:, 128:256]
    full[:, 0:64, 2, :] = xT[:, 256:320]
    full[:, 64:128, 2, :] = xT[:, 256:320]   # row-group duplicate
    xt_main = np.ascontiguousarray(
        full.reshape(B, 128, 3, 4, CHUNK).transpose(0, 3, 1, 2, 4))
    Wp = np.zeros((384, 384), np.float32)
    Wp[0:320, 0:320] = W
    Wp[0:320, 320:384] = W[:, 256:320]       # e-col duplicate -> u row dup
    wp = np.empty((128, 3, 384), np.float32)
    wp[:, 0, :] = Wp[0:128]
    wp[:, 1, :] = Wp[128:256]
    wp[0:64, 2, :] = Wp[256:320]
    wp[64:128, 2, :] = Wp[256:320]           # row-group duplicate
    return xt_main, wp


def _sigmoid(z):
    return 1.0 / (1.0 + np.exp(-z, dtype=np.float32))


def kernel(x, W, b, _trace=False):
    global last_results
    x = np.ascontiguousarray(np.asarray(x, dtype=np.float32))
    W = np.asarray(W, dtype=np.float32)
    b = np.asarray(b, dtype=np.float32)
    bias_val = float(b[0])

    if bias_val not in _cache:
        _cache.clear()
        _cache[bias_val] = _build(bias_val)
    nc = _cache[bias_val]

    xt_main, wp = _host_pack(x, W)
    in_maps = [{"xt_main": xt_main[c], "w": wp} for c in range(N_CORES)]
    res = run_bass_kernel_spmd(nc, in_maps, core_ids=list(range(N_CORES)),
                               trace=_trace)
    last_results = res

    out = np.empty((B, L, L), dtype=np.float32)
    for c in range(N_CORES):
        out[c, :JM, :JM] = np.asarray(res.results[c]["out"]).astype(np.float32)
    # tails on host: rows 2048:2050 (all j) and cols 2048:2050 (i < 2048)
    xt2 = x[:, JM:L, :]                       # (B, 2, 320)
    u2 = np.matmul(xt2, W)                    # (B, 2, 320)
    out[:, JM:, :] = _sigmoid(
        np.matmul(u2, x.transpose(0, 2, 1)) + bias_val)
    v2 = np.matmul(W, xt2.transpose(0, 2, 1))  # (B, 320, 2)
    out[:, :JM, JM:] = _sigmoid(np.matmul(x[:, :JM], v2) + bias_val)
    return out
